# revision 55
# baseline (speedup 1.0000x reference)
"""Self-contained Trainium2 Bass kernel for the 3-layer GAT + graph readout
(nn_GAT_36361193128013). 8-core SPMD over one trn2 chip.

Structure (v8 -- ap_gather a_d, head-minor packing, software-pipelined
windows: 50-window straight-line layer bodies with gathers issued SKEW=4 windows ahead
of their consumers, loads on the SP DMA queue and result writes on the
ACT DMA queue so neither head-blocks the other, per-layer Shared-address
AllGather tables, and a per-window arithmetic blend (c1 + m*(c0-c1))
replacing the int-mask select):
- graph-aligned node sharding (64 graphs / ~6250 nodes per core) so the
  graph readout never crosses cores;
- per-layer node table [h bf16(64, head-minor (c,h)) | a_s f32(4) | pad]
  as 256B rows, AllGather-replicated across the 8 cores;
- the per-edge a_d term no longer uses a one-hot select-reduce (that was
  ~29us/window on DVE at 1x): host packs edge slots so every aligned
  16-slot run (one GPSIMD Q7 core's partitions) touches <=2 distinct dst
  nodes; ONE gpsimd.ap_gather per layer then fetches both candidate a_d
  rows per (tile, q7) run from a broadcast-replicated a_d table; the raw
  candidate pairs are staged to DRAM, streamed per window, and a cheap
  arithmetic blend with a host mask picks the right candidate per slot;
- attention dots (a_s, a_d per node) are fused into the dense projection
  matmuls: rhs = [W | W@as | W@ad] ([*, 72]), so the node phase writes
  table rows + the a_d table straight from PSUM;
- per-edge source-row gather stays gpsimd.dma_gather (wrap16 int16 over
  two table halves, <=768 descriptors/instruction, 4 SWDGE queues);
- edge phase per window: one-hot S3 (is_equal), exp without
  max-subtraction, alpha-weighted scatter via one-hot matmuls
  accumulating [128 dst, 64ch + 4 denom] in PSUM;
- head-minor (c,h) channel order keeps the big DVE multiplies packed
  (2x mode) instead of stride-0-broadcast (1x);
- graph readout via resettable segmented scans + indirect extraction.

kernel(**inputs) takes the FULL inputs and returns [512, 1] float32.
"""
import numpy as np
import ml_dtypes
import concourse.bacc as bacc
import concourse.bass as bass
import concourse.mybir as mybir
import concourse.tile as tile
from concourse.bass import ts
from concourse.bass_utils import run_bass_kernel_spmd

dt = mybir.dt
AF = mybir.ActivationFunctionType
ALU = mybir.AluOpType
AX = mybir.AxisListType

H, C = 4, 16
HC = H * C
N = 50000
G = 512
NC = 8
GPC = G // NC          # graphs per core
W = 128                # dst nodes per window
NLOC = 6400            # padded local nodes per core (multiple of 128)
NWIN = NLOC // W       # 50
TTW = 38               # gather tiles per window (adaptive via OverflowError)
NTAB = NC * NLOC       # 51200 table rows
HALF = NTAB // 2       # int16 dma_gather indices cover one half each
PAD_DSTREL = 200.0
import os
SKEW = int(os.environ.get("K_SKEW", "4"))
HBUFS = int(os.environ.get("K_HBUFS", "6"))
ABLATE = "full"   # timing ablation: full|noag|nogather|noapg|nocompute|shell

# head-minor permutation: new channel ch=(c,h) -> old channel h*16+c
PERM = np.array([(ch % H) * C + ch // H for ch in range(HC)], np.int64)


def pack_groups(counts, t_half):
    """Per (window, half): place each dst's edge run so every aligned
    16-slot block touches <=2 distinct dsts (GPSIMD Q7 cores share
    ap_gather indices across their 16 partitions). Large dsts first.
    Returns (start[NWIN,2,128], d1, d2 [NWIN,2,t_half*8], maxneed)."""
    NW = counts.shape[0]
    hcap = t_half * 128
    nrun = t_half * 8
    big = (int(counts.sum(axis=(2,)).max()) // 16 + 130)  # slack buffer
    start = np.zeros((NW, 2, 128), np.int32)
    d1 = np.zeros((NW, 2, nrun), np.int16)
    d2 = np.zeros((NW, 2, nrun), np.int16)
    maxneed = 0
    for w_ in range(NW):
        for hf in range(2):
            cnt = counts[w_, hf]
            order = np.argsort(-cnt, kind="stable")
            pos = 0
            nd = np.zeros(big, np.int8)
            d1b = np.zeros(big, np.int16)
            d2b = np.zeros(big, np.int16)
            for d in order:
                k = int(cnt[d])
                if k == 0:
                    break
                r = pos >> 4
                if (pos & 15) and nd[r] >= 2:
                    pos = (r + 1) << 4
                start[w_, hf, d] = pos
                e = pos + k
                r0 = pos >> 4
                r1 = (e - 1) >> 4
                if nd[r0] == 0:
                    d1b[r0] = d
                    nd[r0] = 1
                else:
                    d2b[r0] = d
                    nd[r0] = 2
                if r1 > r0:
                    d1b[r0 + 1 : r1 + 1] = d
                    nd[r0 + 1 : r1 + 1] = 1
                pos = e
            maxneed = max(maxneed, pos)
            if pos <= hcap:
                d1[w_, hf] = d1b[:nrun]
                d2[w_, hf] = d2b[:nrun]
    return start, d1, d2, maxneed


def prep(x, edge_index, batch_index, tt):
    """Vectorized host-side prep: per-core node shards + per-window edge
    slot tables (wrap16 int16 gather indices over two table halves).
    Returns (per-core input dicts, gstart)."""
    src = np.asarray(edge_index[0], dtype=np.int64)
    dst = np.asarray(edge_index[1], dtype=np.int64)
    bi = np.asarray(batch_index, dtype=np.int64)
    x = np.asarray(x)
    t_half = tt // 2
    hcap = t_half * 128

    gstart = np.searchsorted(bi, np.arange(0, G + 1, GPC))  # node start per core
    core_of_node = np.searchsorted(gstart, np.arange(N), side="right") - 1
    row_of = core_of_node * NLOC + (np.arange(N) - gstart[core_of_node])

    core_of_edge = np.searchsorted(gstart, dst, side="right") - 1

    inputs = []
    for c in range(NC):
        ns, ne = gstart[c], gstart[c + 1]
        nloc = ne - ns
        m = core_of_edge == c
        e_dst = dst[m] - ns
        e_row = row_of[src[m]]
        # self loops for ALL padded rows (pad nodes project to h=0 -> e=0,
        # exp=1) so every dst's softmax denominator is > 0 and the node
        # phase can use a plain divide without an epsilon guard
        e_dst = np.concatenate([e_dst, np.arange(NLOC)])
        e_row = np.concatenate([e_row, c * NLOC + np.arange(NLOC)])

        win = e_dst >> 7
        drel = e_dst & 127
        is_hi = (e_row >= HALF).astype(np.int64)

        counts = np.zeros((NWIN, 2, 128), np.int32)
        np.add.at(counts, (win, is_hi, drel), 1)
        startp, d1, d2, maxneed = pack_groups(counts, t_half)
        if maxneed > hcap:
            raise OverflowError(2 * maxneed)

        # slot of each edge: start of its (win,half,dst) group + rank inside
        key = (win * 2 + is_hi) * 128 + drel
        order = np.argsort(key, kind="stable")
        key_s = key[order]
        first = np.r_[0, np.flatnonzero(np.diff(key_s)) + 1]
        gidx_of = np.searchsorted(key_s[first], key_s)
        rank = np.arange(len(key_s)) - first[gidx_of]
        slot = startp[win[order], is_hi[order], drel[order]] + rank

        gidx = np.zeros((NWIN, 2, hcap), np.int16)       # pad -> row 0 of half
        drl = np.full((NWIN, 2, hcap), PAD_DSTREL, np.float32)
        rows_s = e_row[order] - is_hi[order] * HALF
        gidx[win[order], is_hi[order], slot] = rows_s
        drl[win[order], is_hi[order], slot] = drel[order]

        # select mask: 1.0 where slot's dst == d1 of its 16-run
        d1rep = np.repeat(d1.reshape(NWIN, 2, t_half * 8), 16, axis=2)
        m1 = (drl == d1rep).astype(np.float32)

        # wrap16: half-list element k sits at [k % 16, k // 16]
        egidx = np.ascontiguousarray(
            gidx.reshape(NWIN, 2, t_half * 8, 16).transpose(0, 1, 3, 2)
        ).reshape(NWIN, 2, 16, t_half * 8)
        egidx = np.tile(egidx, (1, 1, 8, 1)).reshape(NWIN, 2, 128, t_half * 8)
        egidx = np.ascontiguousarray(
            egidx.transpose(2, 0, 1, 3)
        ).reshape(128, NWIN * tt * 8)

        # slot k = t*128 + p -> [p, t]; lo tiles then hi tiles
        def to_pt(a, dtp):
            return (
                np.ascontiguousarray(a.reshape(NWIN, tt, 128).transpose(2, 0, 1))
                .astype(dtp)
                .reshape(128, NWIN * tt)
            )
        dm = np.stack([drl.reshape(NWIN, tt, 128), m1.reshape(NWIN, tt, 128)], 1)
        edm = (
            np.ascontiguousarray(dm.transpose(3, 0, 1, 2))
            .astype(ml_dtypes.bfloat16)
            .reshape(128, NWIN * 2 * tt)
        )

        # ap_gather index list: run (w, hf, r) -> q7 core r%8, global tile
        # hf*t_half + r//8; entries j=(w*TT+t)*2+k2, value = local node id
        d12 = np.stack([d1, d2], axis=-1).astype(np.int32)  # [NW, 2, nrun, 2]
        d12 = d12 + (np.arange(NWIN, dtype=np.int32) * 128)[:, None, None, None]
        d12 = d12.reshape(NWIN, 2, t_half, 8, 2)
        A = np.ascontiguousarray(d12.transpose(3, 0, 1, 2, 4)).reshape(
            8, NWIN * tt * 2
        )
        J = NWIN * tt * 2
        Ccol = -(-J // 16)
        Ap = np.zeros((8, Ccol * 16), np.int32)
        Ap[:, :J] = A
        apidx = (
            np.ascontiguousarray(Ap.reshape(8, Ccol, 16).transpose(0, 2, 1))
            .reshape(128, Ccol)
            .astype(np.int16)
        )

        xT = np.zeros((128, NLOC), ml_dtypes.bfloat16)
        xT[:, :nloc] = x[ns:ne].T.astype(ml_dtypes.bfloat16)

        # readout: graph boundaries within the core
        bounds = np.searchsorted(bi, np.arange(c * GPC, (c + 1) * GPC + 1)) - ns
        z = np.ones(NLOC, np.float32)
        r = np.zeros(NLOC, np.float32)
        z[bounds[:-1]] = 0.0
        r[bounds[:-1]] = -1e30
        z16 = z.reshape(1, NLOC).astype(ml_dtypes.bfloat16)
        r16 = r.reshape(1, NLOC).astype(ml_dtypes.bfloat16)
        gends = (bounds[1:] - 1).astype(np.int32).reshape(GPC, 1)
        cnt = np.diff(bounds).astype(np.float32)
        invcnt64 = (1.0 / np.maximum(cnt, 1.0)).astype(np.float32).reshape(GPC, 1)

        inputs.append(
            dict(xT1=xT, egidx=egidx, edm=edm, apidx=apidx,
                 z16=z16, r16=r16, gends=gends, invcnt64=invcnt64)
        )
    return inputs, gstart


def null_input_decls():
    """Inputs the timing-floor null kernel should also upload (largest bufs)."""
    return [
        ("xT1", [128, NLOC], dt.bfloat16),
        ("egidx", [128, NWIN * TTW * 8], dt.int16),
        ("edm", [128, NWIN * 2 * TTW], dt.bfloat16),
    ]


_ctr = [0]


def split_waits(nc):
    """Walrus codegen only supports one wait per instruction; split extras
    onto NoOps."""
    for _name, bbwrap in nc.bb_map.items():
        bb = bbwrap.bb if hasattr(bbwrap, "bb") else bbwrap
        insts = bb.instructions
        i = 0
        while i < len(insts):
            inst = insts[i]
            si = inst.sync_info
            if si is not None and si.on_wait and len(si.on_wait) > 1:
                waits = list(si.on_wait)
                si.on_wait = waits[:1]
                rest = waits[1:]
                for w in rest:
                    _ctr[0] += 1
                    nop = mybir.InstNoOp(name=f"splitw-{_ctr[0]}", ins=[], outs=[])
                    nop.engine = inst.engine
                    nop.sync_info = mybir.SyncInfo(on_wait=[w], on_update=[])
                    nc.register_instruction(nop)
                    insts.insert(i, nop)
                    i += 1
            i += 1


def build(n_cores=8, tt=TTW, reps=1):
    TT = tt
    JADE = NWIN * TT * 2           # ap_gather num_idxs
    CAP = -(-JADE // 16)           # apidx columns
    nc = bacc.Bacc(target_bir_lowering=False, num_swdge_queues=4)

    xT1 = nc.declare_dram_parameter("xT1", [128, NLOC], dt.bfloat16, isOutput=False)
    egidxd = nc.declare_dram_parameter("egidx", [128, NWIN * TT * 8], dt.int16, isOutput=False)
    edmd = nc.declare_dram_parameter("edm", [128, NWIN * 2 * TT], dt.bfloat16, isOutput=False)
    apidxd = nc.declare_dram_parameter("apidx", [128, CAP], dt.int16, isOutput=False)
    z16d = nc.declare_dram_parameter("z16", [1, NLOC], dt.bfloat16, isOutput=False)
    r16d = nc.declare_dram_parameter("r16", [1, NLOC], dt.bfloat16, isOutput=False)
    gendd = nc.declare_dram_parameter("gends", [64, 1], dt.int32, isOutput=False)
    invcd = nc.declare_dram_parameter("invcnt64", [64, 1], dt.float32, isOutput=False)
    Wd = {
        1: nc.declare_dram_parameter("W1cat", [128, 72], dt.bfloat16, isOutput=False),
        2: nc.declare_dram_parameter("W2cat", [64, 72], dt.bfloat16, isOutput=False),
        3: nc.declare_dram_parameter("W3cat", [64, 72], dt.bfloat16, isOutput=False),
    }
    bd = {}
    for l in (1, 2, 3):
        bd[l] = nc.declare_dram_parameter(f"brep{l}", [1, 64], dt.float32, isOutput=False)
    iotad = nc.declare_dram_parameter("iota", [1, 128], dt.bfloat16, isOutput=False)
    idf32d = nc.declare_dram_parameter("idf32", [128, 128], dt.float32, isOutput=False)
    wmaxd = nc.declare_dram_parameter("wmaxr", [64, 16], dt.float32, isOutput=False)
    wmeand = nc.declare_dram_parameter("wmeanr", [64, 16], dt.float32, isOutput=False)
    wsumd = nc.declare_dram_parameter("wsumr", [64, 16], dt.float32, isOutput=False)
    boutd = nc.declare_dram_parameter("boutr", [64, 1], dt.float32, isOutput=False)
    outd = nc.declare_dram_parameter("out", [64, 1], dt.float32, isOutput=True)

    with tile.TileContext(nc) as tc:
      for rep in range(reps):
        with tc.tile_pool(name=f"{rep}_dram", bufs=1, space="DRAM") as dp:
          # DRAM internals (outlive both phases)
          myrows = dp.tile([NLOC, 128], dt.bfloat16, tag="myrows")
          tables = [
              dp.tile([NTAB, 128], dt.bfloat16, tag=f"table{i}",
                      name=f"table{i}", addr_space="Shared")
              for i in (1, 2, 3)
          ]
          adtabTd = dp.tile([NLOC, 4], dt.bfloat16, tag="adtabT")
          adeAllD = dp.tile([128, NWIN * TT * 8], dt.bfloat16, tag="adeAllD")
          hmD = dp.tile([16, NLOC], dt.float32, tag="hmD")
          gsD = dp.tile([NLOC, 16], dt.float32, tag="gsD")
          gmD = dp.tile([NLOC, 16], dt.float32, tag="gmD")
          with (
              tc.tile_pool(name=f"{rep}_const", bufs=1) as cp,
              tc.tile_pool(name=f"{rep}_hsrc", bufs=HBUFS) as hp,
              tc.tile_pool(name=f"{rep}_lds", bufs=8) as lp,
              tc.tile_pool(name=f"{rep}_work", bufs=3) as wp,
              tc.tile_pool(name=f"{rep}_sml", bufs=4) as sp,
              tc.tile_pool(name=f"{rep}_ps_acc", bufs=4, space="PSUM") as pa,
              tc.tile_pool(name=f"{rep}_ps_misc", bufs=2, space="PSUM") as pm_,
          ):
              def ld(dram, shape, dtp):
                  t = cp.tile(shape, dtp, tag=dram.name)
                  nc.sync.dma_start(t[:], dram[:])
                  return t

              def ldb(dram, shape, dtp):
                  # single-row param broadcast to all partitions at load time
                  t = cp.tile(shape, dtp, tag=dram.name)
                  nc.sync.dma_start(t[:], dram[0:1, :].to_broadcast(shape))
                  return t

              Ws = {l: ld(Wd[l], Wd[l].shape, dt.bfloat16) for l in (1, 2, 3)}
              bs = {l: ldb(bd[l], [128, 64], dt.float32) for l in (1, 2, 3)}
              iota = ldb(iotad, [128, 128], dt.bfloat16)
              idf32 = ld(idf32d, [128, 128], dt.float32)
              apidx = ld(apidxd, [128, CAP], dt.int16)

              def table_row_write(iv, ps):
                  """Pack node-table row [h bf16 | as f32] from PSUM [128,72]
                  (h cols 0:64, as 64:68, ad 68:72) and write myrows+adtabT."""
                  rowt = sp.tile([128, 128], dt.bfloat16, tag="rowt")
                  nc.vector.tensor_copy(rowt[:, 0:64], ps[:, 0:64])
                  nc.vector.tensor_copy(rowt[:, 64:72].bitcast(dt.float32), ps[:, 64:68])
                  nc.scalar.dma_start(myrows[ts(iv, 128), :], rowt[:])
                  ad_bf = sp.tile([128, 4], dt.bfloat16, tag="ad_bf")
                  nc.vector.tensor_copy(ad_bf[:], ps[:, 68:72])
                  nc.scalar.dma_start(adtabTd[ts(iv, 128), :], ad_bf[:])

              def allgather(i):
                  if ABLATE == "noag":
                      return
                  nc.gpsimd.collective_compute(
                      "AllGather",
                      ALU.bypass,
                      replica_groups=[list(range(n_cores))],
                      ins=[myrows[:].opt()],
                      outs=[tables[i][:].opt()],
                  )

              # ---- dense phase, layer 1 ----
              with tc.For_i(0, NWIN, 10) as iv:
                  for dsub in range(10):
                      ivd = iv + dsub
                      xc = wp.tile([128, 128], dt.bfloat16, tag="xc")
                      nc.sync.dma_start(xc[:], xT1[:, ts(ivd, 128)])
                      h_ps = pm_.tile([128, 72], dt.float32, tag="h_ps", space="PSUM")
                      nc.tensor.matmul(out=h_ps[:], lhsT=xc[:], rhs=Ws[1][:], start=True, stop=True)
                      table_row_write(ivd, h_ps)
              def ade_section(l):
                  # a_d gather for layer l: <=2 dsts per 16-slot run, both
                  # candidates fetched by one ap_gather; emitted BEFORE the
                  # AllGather so its DMA traffic hides under the collective
                  with tc.tile_pool(name=f"{rep}_{l}_ade", bufs=1) as ap_:
                      adtabRep = ap_.tile([128, NLOC, 4], dt.bfloat16, tag="adtabRep")
                      nc.sync.dma_start(
                          adtabRep[:],
                          adtabTd[:].unsqueeze(0).to_broadcast([128, NLOC, 4]),
                      )
                      adeAll = ap_.tile([128, JADE, 4], dt.bfloat16, tag="adeAll")
                      if ABLATE in ("noapg", "shell", "gatheronly", "gatherhalf"):
                          nc.vector.memset(adeAll[:], 0.0)
                      else:
                          nc.gpsimd.ap_gather(
                              out_ap=adeAll[:],
                              in_ap=adtabRep[:],
                              idxs_ap=apidx[:],
                              channels=128,
                              num_elems=NLOC,
                              d=4,
                              num_idxs=JADE,
                          )
                      nc.scalar.dma_start(
                          adeAllD[:], adeAll[:].rearrange("p j f -> p (j f)")
                      )

              ade_section(1)
              allgather(0)

              # ---- edge phase per layer (layer l+1 dense fused in) ----
              for l in (1, 2, 3):
                  UN = int(os.environ.get("K_UN", "50"))
                  with tc.For_i(0, NWIN, UN) as iv:
                      stage = {}
                      for k in range(UN + SKEW):
                        if k < UN:
                          # ---- stage A: loads + hsrc gather (SKEW ahead) ----
                          ivx = iv + k
                          edm = lp.tile([128, 2, TT], dt.bfloat16, tag="edm")
                          nc.sync.dma_start(
                              edm[:].rearrange("p a t -> p (a t)"),
                              edmd[:, ts(ivx, 2 * TT)],
                          )
                          drel = edm[:, 0, :]
                          gW = lp.tile([128, TT * 8], dt.int16, tag="gW")
                          nc.sync.dma_start(gW[:], egidxd[:, ts(ivx, TT * 8)])
                          adeW = lp.tile([128, TT, 2, 4], dt.bfloat16, tag="adeW")
                          nc.sync.dma_start(
                              adeW[:].rearrange("p t k f -> p (t k f)"),
                              adeAllD[:, ts(ivx, TT * 8)],
                          )
                          hsrc = hp.tile([128, TT, 128], dt.bfloat16, tag="hsrc")
                          if ABLATE in ("nogather", "shell"):
                              nc.vector.memset(hsrc[:, 0, :], 0.5)
                          else:
                              halves = (((0, 0, HALF),) if ABLATE == "gatherhalf"
                                        else ((0, 0, HALF), (TT // 2, HALF, NTAB)))
                              # 3 near-equal chunks per half (<=896
                              # descriptors each; bigger chunks wedge the
                              # device); round-robin across the 4 SWDGE queues
                              TLO = TT // 2
                              nch = max(1, -(-TLO // 7))
                              cb = [TLO // nch + (i < TLO % nch) for i in range(nch)]
                              q = 3 * k
                              for base, tab_lo, tab_hi in halves:
                                  c0 = 0
                                  for cw in cb:
                                      nc.gpsimd.dma_gather(
                                          out_ap=hsrc[:, base + c0 : base + c0 + cw, :],
                                          in_ap=tables[l - 1][tab_lo:tab_hi, :],
                                          idxs_ap=gW[:, (base + c0) * 8 : (base + c0 + cw) * 8],
                                          num_idxs=cw * 128, num_idxs_reg=cw * 128,
                                          elem_size=128, queue_num=q % 4,
                                      )
                                      q += 1
                                      c0 += cw
                          stage[k] = (edm, adeW, hsrc)
                        if k < SKEW:
                          continue
                        # ---- stage B: compute + node phase (SKEW behind) ----
                        if ABLATE in ("gatheronly", "gatherhalf"):
                          sub = k - SKEW
                          edm, adeW, hsrc = stage.pop(sub)
                          snk = sp.tile([128, 1], dt.bfloat16, tag="snk")
                          nc.vector.tensor_copy(snk[:], hsrc[:, 0, 0:1])
                          continue
                        if True:
                          sub = k - SKEW
                          ivx = iv + sub
                          edm, adeW, hsrc = stage.pop(sub)
                          drel = edm[:, 0, :]
                          S3 = wp.tile([128, TT, W], dt.bfloat16, tag="S3")
                          if ABLATE not in ("nocompute", "shell"):
                              nc.vector.tensor_tensor(
                                  out=S3[:],
                                  in0=drel[:].to_broadcast([128, TT, W]),
                                  in1=iota[:].unsqueeze(1).to_broadcast([128, TT, W]),
                                  op=ALU.is_equal,
                              )
                          adS = sp.tile([128, TT, 4], dt.bfloat16, tag="adS")
                          nc.vector.tensor_tensor(
                              out=adS[:], in0=adeW[:, :, 0, :], in1=adeW[:, :, 1, :],
                              op=ALU.subtract,
                          )
                          nc.vector.tensor_tensor(
                              out=adS[:], in0=adS[:],
                              in1=edm[:, 1, :].unsqueeze(2).to_broadcast([128, TT, 4]),
                              op=ALU.mult,
                          )
                          nc.vector.tensor_add(adS[:], adS[:], adeW[:, :, 1, :])
                          e_sb = sp.tile([128, TT, 4], dt.float32, tag="e_sb")
                          nc.vector.tensor_tensor(
                              out=e_sb[:],
                              in0=hsrc[:, :, 64:72].bitcast(dt.float32),
                              in1=adS[:],
                              op=ALU.add,
                          )
                          nc.vector.scalar_tensor_tensor(
                              out=e_sb[:], in0=e_sb[:], scalar=0.2, in1=e_sb[:],
                              op0=ALU.mult, op1=ALU.max,
                          )
                          wmsg = wp.tile([128, TT, 72], dt.bfloat16, tag="wmsg")
                          out_ps = pa.tile([128, 68], dt.float32, tag="out_ps", space="PSUM")
                          if ABLATE in ("nocompute", "shell"):
                              nc.vector.memset(out_ps[:], 1.0)
                          else:
                              nc.scalar.activation(
                                  wmsg[:, :, 64:68], e_sb[:], AF.Exp,
                              )
                              # head-minor: exp broadcast lands on (c) dim,
                              # innermost h stays packed -> 2x DVE mode
                              nc.vector.tensor_tensor(
                                  out=wmsg[:, :, 0:64].rearrange("p t (c h) -> p t c h", h=4),
                                  in0=hsrc[:, :, 0:64].rearrange("p t (c h) -> p t c h", h=4),
                                  in1=wmsg[:, :, 64:68].unsqueeze(2).to_broadcast([128, TT, 16, 4]),
                                  op=ALU.mult,
                              )
                              for t in range(TT):
                                  nc.tensor.matmul(
                                      out=out_ps[:],
                                      lhsT=S3[:, t, :],
                                      rhs=wmsg[:, t, 0:68],
                                      start=(t == 0), stop=(t == TT - 1),
                                  )
                          # node phase: denom > 0 guaranteed by self loops
                          rs = sp.tile([128, 4], dt.float32, tag="rs")
                          nc.vector.reciprocal(rs[:], out_ps[:, 64:68])
                          xn = sp.tile([128, 64], dt.float32, tag="xn")
                          nc.vector.tensor_tensor(
                              out=xn[:].rearrange("p (c h) -> p c h", h=4),
                              in0=out_ps[:, 0:64].rearrange("p (c h) -> p c h", h=4),
                              in1=rs[:].unsqueeze(1).to_broadcast([128, 16, 4]),
                              op=ALU.mult,
                          )
                          nc.vector.tensor_add(xn[:], xn[:], bs[l][:])
                          nc.scalar.activation(xn[:], xn[:], AF.Tanh)
                          if l < 3:
                              # fused dense for layer l+1
                              xt_ps = pm_.tile([64, 128], dt.float32, tag="xt_ps", space="PSUM")
                              nc.tensor.transpose(out=xt_ps[:], in_=xn[:], identity=idf32[:])
                              xt_sb = sp.tile([64, 128], dt.bfloat16, tag="xt_sb")
                              nc.vector.tensor_copy(xt_sb[:], xt_ps[:])
                              h2_ps = pm_.tile([128, 72], dt.float32, tag="h_ps", space="PSUM")
                              nc.tensor.matmul(
                                  out=h2_ps[:], lhsT=xt_sb[:], rhs=Ws[l + 1][:],
                                  start=True, stop=True,
                              )
                              table_row_write(ivx, h2_ps)
                          else:
                              hm = sp.tile([128, 16], dt.float32, tag="hm")
                              nc.vector.tensor_reduce(
                                  hm[:], xn[:].rearrange("p (c h) -> p c h", h=4),
                                  axis=AX.X, op=ALU.add,
                              )
                              hm_ps = pm_.tile([16, 128], dt.float32, tag="xt_ps", space="PSUM")
                              nc.tensor.transpose(out=hm_ps[:], in_=hm[:], identity=idf32[:])
                              hm_sb = sp.tile([16, 128], dt.float32, tag="hm_sb")
                              nc.vector.tensor_copy(hm_sb[:], hm_ps[:])
                              nc.scalar.dma_start(hmD[:, ts(ivx, 128)], hm_sb[:])
                  if l < 3:
                      ade_section(l + 1)
                      allgather(l)

          # ---- readout (own pool scope; SBUF from the layer phase is freed) ----
          with (
              tc.tile_pool(name=f"{rep}_ro", bufs=1) as cp,
              tc.tile_pool(name=f"{rep}_ros", bufs=2) as sp,
              tc.tile_pool(name=f"{rep}_rop", bufs=2, space="PSUM") as pm_,
          ):
              idro = cp.tile([16, 16], dt.float32, tag="idro")
              nc.sync.dma_start(idro[:], idf32d[0:16, 0:16])
              hmT = cp.tile([16, NLOC], dt.float32, tag="hmT")
              if ABLATE in ("gatheronly", "gatherhalf"):
                  nc.vector.memset(hmT[:], 0.0)
              else:
                  nc.sync.dma_start(hmT[:], hmD[:])
              z16 = cp.tile([16, NLOC], dt.bfloat16, tag="z16")
              r16 = cp.tile([16, NLOC], dt.bfloat16, tag="r16")
              nc.sync.dma_start(z16[:], z16d[0:1, :].to_broadcast([16, NLOC]))
              nc.sync.dma_start(r16[:], r16d[0:1, :].to_broadcast([16, NLOC]))
              gsumT = cp.tile([16, NLOC], dt.float32, tag="gsumT")
              gmaxT = cp.tile([16, NLOC], dt.float32, tag="gmaxT")
              nc.vector.tensor_tensor_scan(
                  out=gsumT[:], data0=z16[:], data1=hmT[:], initial=0.0,
                  op0=ALU.mult, op1=ALU.add,
              )
              nc.vector.tensor_tensor_scan(
                  out=gmaxT[:], data0=r16[:], data1=hmT[:], initial=-1e30,
                  op0=ALU.add, op1=ALU.max,
              )
              for w in range(NWIN):
                  for (scanT, stage, tg) in ((gsumT, gsD, "s"), (gmaxT, gmD, "m")):
                      tp = pm_.tile([128, 16], dt.float32, tag="rops" + tg, space="PSUM")
                      nc.tensor.transpose(
                          out=tp[:], in_=scanT[:, w * 128 : (w + 1) * 128],
                          identity=idro[:],
                      )
                      tsb = sp.tile([128, 16], dt.float32, tag="tsb" + tg)
                      nc.vector.tensor_copy(tsb[:], tp[:])
                      nc.sync.dma_start(stage[w * 128 : (w + 1) * 128, :], tsb[:])

              wmax = cp.tile([64, 16], dt.float32, tag="wmax")
              wmean = cp.tile([64, 16], dt.float32, tag="wmean")
              wsum = cp.tile([64, 16], dt.float32, tag="wsum")
              bout = cp.tile([64, 1], dt.float32, tag="bout")
              gend = cp.tile([64, 1], dt.int32, tag="gend")
              invc = cp.tile([64, 1], dt.float32, tag="invc")
              nc.sync.dma_start(wmax[:], wmaxd[:])
              nc.sync.dma_start(wmean[:], wmeand[:])
              nc.sync.dma_start(wsum[:], wsumd[:])
              nc.sync.dma_start(bout[:], boutd[:])
              nc.sync.dma_start(gend[:], gendd[:])
              nc.sync.dma_start(invc[:], invcd[:])
              gsE = sp.tile([64, 16], dt.float32, tag="gsE")
              gmE = sp.tile([64, 16], dt.float32, tag="gmE")
              nc.gpsimd.indirect_dma_start(
                  out=gsE[:], out_offset=None, in_=gsD[:],
                  in_offset=bass.IndirectOffsetOnAxis(ap=gend[:], axis=0),
              )
              nc.gpsimd.indirect_dma_start(
                  out=gmE[:], out_offset=None, in_=gmD[:],
                  in_offset=bass.IndirectOffsetOnAxis(ap=gend[:], axis=0),
              )
              acc = sp.tile([64, 16], dt.float32, tag="acc")
              tmp2 = sp.tile([64, 16], dt.float32, tag="tmp2")
              # acc = gmax*wmax + gsum*wsum + gsum*invc*wmean  (x0.25 at the end)
              nc.vector.tensor_mul(acc[:], gmE[:], wmax[:])
              nc.vector.tensor_mul(tmp2[:], gsE[:], wsum[:])
              nc.vector.tensor_add(acc[:], acc[:], tmp2[:])
              nc.vector.tensor_mul(tmp2[:], gsE[:], wmean[:])
              nc.vector.tensor_mul(tmp2[:], tmp2[:], invc[:].to_broadcast([64, 16]))
              nc.vector.tensor_add(acc[:], acc[:], tmp2[:])
              osum = sp.tile([64, 1], dt.float32, tag="osum")
              nc.vector.tensor_reduce(osum[:], acc[:], axis=AX.X, op=ALU.add)
              o_sb = sp.tile([64, 1], dt.float32, tag="o_sb")
              nc.vector.tensor_scalar_mul(o_sb[:], osum[:], 0.25)
              nc.vector.tensor_add(o_sb[:], o_sb[:], bout[:])
              nc.sync.dma_start(outd[:], o_sb[:])

    nc.compile()
    split_waits(nc)
    return nc


def prep_params(d):
    """Replicated parameter tensors (same for all cores). Weight matrices
    are fused [W | W@as | W@ad] with head-minor (c,h) channel order."""
    out = {}
    out["iota"] = np.arange(W, dtype=np.float32).reshape(1, W).astype(
        ml_dtypes.bfloat16
    )
    out["idf32"] = np.eye(128, dtype=np.float32)
    for l, fin in ((1, 128), (2, HC), (3, HC)):
        Wl = np.asarray(d[f"W{l}"], np.float32)
        asl = np.asarray(d[f"as{l}"], np.float32)   # [H, C]
        adl = np.asarray(d[f"ad{l}"], np.float32)
        Was = np.stack([Wl[:, h * C : (h + 1) * C] @ asl[h] for h in range(H)], 1)
        Wad = np.stack([Wl[:, h * C : (h + 1) * C] @ adl[h] for h in range(H)], 1)
        Wp = Wl[:, PERM]                             # head-minor columns
        if l > 1:
            Wp = Wp[PERM, :]                         # head-minor rows too
            Was = Was[PERM, :]
            Wad = Wad[PERM, :]
        out[f"W{l}cat"] = np.concatenate([Wp, Was, Wad], 1).astype(ml_dtypes.bfloat16)
        out[f"brep{l}"] = np.asarray(d[f"b{l}"], np.float32)[PERM].reshape(1, HC)
    Wout = np.asarray(d["Wout"], np.float32)
    out["wmaxr"] = np.tile(Wout[0:16].reshape(1, 16), (64, 1))
    out["wmeanr"] = np.tile(Wout[16:32].reshape(1, 16), (64, 1))
    out["wsumr"] = np.tile(Wout[32:48].reshape(1, 16), (64, 1))
    out["boutr"] = np.full((64, 1), np.float32(np.asarray(d["bout"]).reshape(-1)[0]))
    return out


def make_in_maps(d, tt=TTW):
    inputs, _ = prep(d["x"], d["edge_index"], d["batch_index"], tt)
    params = prep_params(d)
    maps = []
    for c in range(NC):
        m = dict(inputs[c])
        m.update(params)
        maps.append(m)
    return maps


_CACHE = {}


def kernel(**inputs) -> np.ndarray:
    d = {k: np.asarray(v) for k, v in inputs.items()}
    tt = TTW
    while True:
        try:
            maps = make_in_maps(d, tt)
            break
        except OverflowError as e:
            tt = 2 * (-(-int(e.args[0]) // 256))
    if tt not in _CACHE:
        _CACHE[tt] = build(NC, tt)
    nc = _CACHE[tt]
    res = run_bass_kernel_spmd(nc, maps, list(range(NC)))
    got = np.concatenate([res.results[c]["out"].reshape(-1) for c in range(NC)])
    return got.reshape(G, 1).astype(np.float32)
